# revision 2
# baseline (speedup 1.0000x reference)
"""Multi-head attention (RoPE + causal softmax + out-proj) on 8 TRN2 NeuronCores.

Sharding: core c handles batch b = c // 2 and head-half g = c % 2 (8 of 16
heads). Each core computes q/k/v projections for its heads, RoPE, causal
attention, and a partial transposed output projection
outT = (y_heads @ Wo_part.T).T; the host sums the two partials per batch.

Device layout notes:
 - All matmul operands are float32r (single-pass full-rate PE fp32).
 - q/k weight rows are permuted host-side so the RoPE even/odd pairs become
   contiguous 32-partition blocks: tiles hold [x1 of 4 heads | x2 of 4 heads]
   and RoPE runs as full-width vector ops.  Attention contracts over dh=64 as
   two K=32 matmul passes (x1, x2) per head; two heads run concurrently on
   distinct 32-row PE groups via tile_position.
 - Attention is k-major: sT = k q^T tiles [k:128, q:512]; exp on ScalarE
   (1/sqrt(dh) folded into the activation scale); causal handling is
   tile-level skipping plus a precomputed 0/1 mask multiply (on GPSIMD) for
   diagonal tiles; y^T = v_aug.T @ expT where v_aug carries a ones column
   per head, making row 64 of y^T the softmax denominator for free.
 - Normalization is per-head-pair (reciprocal chunks + K=1 PE broadcast
   matmul + DVE multiply) so it overlaps the next pair's attention.
"""

import numpy as np

B, T, C, H = 4, 2048, 1024, 16
DH = C // H  # 64
NCORES = 8
HPC = H // 2  # 8 heads per core
QR = HPC * DH  # 512 rows per q/k/v section
TS = 512  # t-chunk width
NTS = T // TS  # 4
CC = C // 128  # 8 contraction chunks
NKT = T // 128  # 16 k-tiles / t-row chunks

_CACHE = {}


def _build_program():
    import concourse.mybir as mybir
    import concourse.tile as tile
    from concourse import bacc

    f32 = mybir.dt.float32
    f32r = mybir.dt.float32r
    EXP = mybir.ActivationFunctionType.Exp

    nc = bacc.Bacc(trn_type="TRN2")

    xT = nc.dram_tensor("xT", [C, T], f32, kind="ExternalInput").ap()
    wqkvT = nc.dram_tensor("wqkvT", [C, 3 * QR], f32, kind="ExternalInput").ap()
    woT = nc.dram_tensor("woT", [QR, C], f32, kind="ExternalInput").ap()
    cosT = nc.dram_tensor("cosT", [128, T], f32, kind="ExternalInput").ap()
    sinT = nc.dram_tensor("sinT", [128, T], f32, kind="ExternalInput").ap()
    maskd = nc.dram_tensor("maskd", [128, TS + 128], f32, kind="ExternalInput").ap()
    outT = nc.dram_tensor("outT", [C, T], f32, kind="ExternalOutput").ap()

    with tile.TileContext(nc) as tc:
        with tc.tile_pool(name="persist", bufs=1) as pp:
            # rope'd q/k in projection layout: key (part, grp, half):
            # rows 32*i = x-half of local head 4*grp+i
            qk = {}
            for part in range(2):
                for grp in range(2):
                    for half in range(2):
                        nm = f"qk{part}{grp}{half}"
                        qk[(part, grp, half)] = pp.tile(
                            [128, T], f32r, tag=nm, name=nm
                        )
            # v with a ones column per head: [t-chunk 128, 8 * 65]
            v_aug = [
                pp.tile([128, HPC * 65], f32r, tag=f"va{t}", name=f"va{t}")
                for t in range(NKT)
            ]

            # ---------------- phase A: qkv projection + rope ----------------
            with (
                tc.tile_pool(name="wpool", bufs=1) as wp,
                tc.tile_pool(name="xpool", bufs=9) as xp,
                tc.tile_pool(name="trig", bufs=1) as tp,
                tc.tile_pool(name="ropetmp", bufs=6) as rt,
                tc.tile_pool(name="psA", bufs=4, space="PSUM") as psA,
            ):
                # first weight chunk, then first x chunk set, then the rest —
                # so the first matmul group isn't stuck behind 6 MB of weights
                wtiles = [None] * CC
                w = wp.tile([128, 3 * QR], f32r, tag="w0", name="w0")
                nc.sync.dma_start(w[:], wqkvT[0:128, :].bitcast(f32r))
                wtiles[0] = w
                xts0 = []
                for cc in range(CC):
                    xt = xp.tile([128, TS], f32r, tag="xts", name=f"x0{cc}")
                    nc.sync.dma_start(
                        xt[:], xT[128 * cc : 128 * (cc + 1), 0:TS].bitcast(f32r)
                    )
                    xts0.append(xt)
                for cc in range(1, CC):
                    w = wp.tile([128, 3 * QR], f32r, tag=f"w{cc}", name=f"w{cc}")
                    nc.sync.dma_start(
                        w[:], wqkvT[128 * cc : 128 * (cc + 1), :].bitcast(f32r)
                    )
                    wtiles[cc] = w
                ct = tp.tile([128, T], f32, tag="cos")
                st_ = tp.tile([128, T], f32, tag="sin")
                nc.sync.dma_start(ct[:], cosT[:])
                nc.sync.dma_start(st_[:], sinT[:])

                ones8 = tp.tile([128, HPC], f32, tag="ones8")
                nc.gpsimd.memset(ones8[:], 1.0)
                for t in range(NKT):
                    var = v_aug[t].rearrange("p (h d) -> p h d", h=HPC)
                    nc.vector.tensor_copy(var[:, :, 64:65], ones8[:].unsqueeze(2))

                for ts in range(NTS):
                    if ts == 0:
                        xts = xts0
                    else:
                        xts = []
                        for cc in range(CC):
                            xt = xp.tile([128, TS], f32r, tag="xts", name=f"x{ts}{cc}")
                            nc.sync.dma_start(
                                xt[:],
                                xT[
                                    128 * cc : 128 * (cc + 1), TS * ts : TS * (ts + 1)
                                ].bitcast(f32r),
                            )
                            xts.append(xt)

                    # q/k rows -> rope (written straight into persistent tiles)
                    for part in range(2):  # 0=q, 1=k
                        for grp in range(2):  # local heads 4*grp .. 4*grp+3
                            ptiles = []
                            for half in range(2):  # x1, x2
                                p = psA.tile(
                                    [128, TS], f32, tag="proj", name=f"p{ts}{part}{grp}{half}"
                                )
                                col0 = QR * part + 256 * grp + 128 * half
                                for cc in range(CC):
                                    nc.tensor.matmul(
                                        p[:],
                                        wtiles[cc][:, col0 : col0 + 128],
                                        xts[cc][:],
                                        start=(cc == 0),
                                        stop=(cc == CC - 1),
                                    )
                                ptiles.append(p)
                            x1p, x2p = ptiles
                            csl = ct[:, TS * ts : TS * (ts + 1)]
                            ssl = st_[:, TS * ts : TS * (ts + 1)]
                            o1 = qk[(part, grp, 0)][:, TS * ts : TS * (ts + 1)]
                            o2 = qk[(part, grp, 1)][:, TS * ts : TS * (ts + 1)]
                            t1 = rt.tile([128, TS], f32, tag="rt", name=f"t1{ts}{part}{grp}")
                            t2 = rt.tile([128, TS], f32, tag="rt", name=f"t2{ts}{part}{grp}")
                            nc.vector.tensor_mul(t1[:], x1p[:], csl)
                            nc.vector.tensor_mul(t2[:], x2p[:], ssl)
                            nc.vector.tensor_sub(o1, t1[:], t2[:])
                            t3 = rt.tile([128, TS], f32, tag="rt", name=f"t3{ts}{part}{grp}")
                            t4 = rt.tile([128, TS], f32, tag="rt", name=f"t4{ts}{part}{grp}")
                            nc.vector.tensor_mul(t3[:], x1p[:], ssl)
                            nc.vector.tensor_mul(t4[:], x2p[:], csl)
                            nc.vector.tensor_add(o2, t3[:], t4[:])

                    # v projection straight into v_aug
                    for tr4 in range(4):
                        t = 4 * ts + tr4
                        p = psA.tile([128, QR], f32, tag="proj", name=f"pv{ts}{tr4}")
                        for cc in range(CC):
                            nc.tensor.matmul(
                                p[:],
                                xts[cc][:, 128 * tr4 : 128 * (tr4 + 1)],
                                wtiles[cc][:, 2 * QR : 3 * QR],
                                start=(cc == 0),
                                stop=(cc == CC - 1),
                            )
                        var = v_aug[t].rearrange("p (h d) -> p h d", h=HPC)
                        nc.vector.tensor_copy(
                            var[:, :, 0:64],
                            p[:].rearrange("p (h d) -> p h d", h=HPC),
                        )

            # ---------------- phase B: attention ----------------
            with tc.tile_pool(name="pB", bufs=1) as pb:
                yT_all = [
                    pb.tile([128, T], f32r, tag=f"ya{j}", name=f"ya{j}")
                    for j in range(4)
                ]
                mt = pb.tile([128, TS + 128], f32r, tag="mask")
                nc.sync.dma_start(mt[:], maskd[:].bitcast(f32r))
                dn = pb.tile([128, TS], f32, tag="dn")
                rcp = pb.tile([128, TS], f32, tag="rcp")

                with (
                    tc.tile_pool(name="epool", bufs=4) as ep,
                    tc.tile_pool(name="dstage", bufs=2) as dsp,
                    tc.tile_pool(name="rstage", bufs=4) as rsp,
                    tc.tile_pool(name="bstage", bufs=4) as bsp,
                    tc.tile_pool(name="psS", bufs=1, space="PSUM") as psS,
                    tc.tile_pool(name="psY", bufs=1, space="PSUM") as psY,
                ):
                    for g4 in range(2):  # head groups of 4: heads 4*g4..4*g4+3
                        for qi in range(NTS):
                            q0 = TS * qi
                            nkt = 4 * (qi + 1)
                            yTs = [
                                psY.tile(
                                    [65, TS], f32, tag=f"yT{i}", name=f"yT{g4}_{qi}_{i}"
                                )
                                for i in range(4)
                            ]
                            for kt in range(nkt):
                                k0 = 128 * kt
                                sT = psS.tile(
                                    [128, 4 * TS], f32, tag="sT", name=f"sT{g4}_{qi}_{kt}"
                                )
                                for lh4 in range(4):
                                    rb = 32 * lh4
                                    for half in range(2):
                                        nc.tensor.matmul(
                                            sT[:, TS * lh4 : TS * (lh4 + 1)],
                                            qk[(1, g4, half)][rb : rb + 32, k0 : k0 + 128],
                                            qk[(0, g4, half)][rb : rb + 32, q0 : q0 + TS],
                                            start=(half == 0),
                                            stop=(half == 1),
                                            tile_position=(rb, 0),
                                        )
                                eT = ep.tile(
                                    [128, 4 * TS], f32r, tag="eT", name=f"eT{g4}_{qi}_{kt}"
                                )
                                nc.scalar.activation(eT[:], sT[:], EXP, scale=0.125)
                                r = kt - 4 * qi
                                if r >= 0:
                                    # causal: cols < 128*r are fully masked,
                                    # then a 128-wide triangular strip.
                                    w = 128 * (r + 1)
                                    msl = mt[:, TS - 128 * r : TS + 128]
                                    ev = eT[:].rearrange("p (s q) -> p s q", s=4)
                                    nc.gpsimd.tensor_mul(
                                        ev[:, :, 0:w],
                                        ev[:, :, 0:w],
                                        msl.unsqueeze(1).broadcast_to([128, 4, w]),
                                    )
                                for lh4 in range(4):
                                    h = 4 * g4 + lh4
                                    nc.tensor.matmul(
                                        yTs[lh4][:],
                                        v_aug[kt][:, 65 * h : 65 * h + 65],
                                        eT[:, TS * lh4 : TS * (lh4 + 1)],
                                        start=(kt == 0),
                                        stop=(kt == nkt - 1),
                                    )
                            # unnormalized copy + denominator staging
                            for lh4 in range(4):
                                h = 4 * g4 + lh4
                                j, e = h // 2, h % 2
                                ridx = 64 * g4 + 4 * lh4 + qi
                                nc.vector.tensor_copy(
                                    yT_all[j][64 * e : 64 * e + 64, q0 : q0 + TS],
                                    yTs[lh4][0:64, :],
                                )
                                dtmp = dsp.tile(
                                    [65, TS], f32, tag="dt", name=f"dt{h}_{qi}"
                                )
                                nc.vector.tensor_copy(dtmp[64:65, :], yTs[lh4][64:65, :])
                                nc.sync.dma_start(
                                    dn[ridx : ridx + 1, :], dtmp[64:65, :]
                                )

                        # per-group normalization (overlaps next group's attention)
                        r0 = 64 * g4
                        nc.vector.reciprocal(
                            rcp[r0 : r0 + 16, :], dn[r0 : r0 + 16, :]
                        )
                        for lh4 in range(4):
                            h = 4 * g4 + lh4
                            j, e = h // 2, h % 2
                            for qi in range(NTS):
                                ridx = 64 * g4 + 4 * lh4 + qi
                                q0 = TS * qi
                                rtile = rsp.tile(
                                    [1, TS], f32, tag="rr", name=f"rr{h}_{qi}"
                                )
                                nc.sync.dma_start(rtile[:], rcp[ridx : ridx + 1, :])
                                bcS = bsp.tile(
                                    [128, TS], f32, tag="bb", name=f"bb{h}_{qi}"
                                )
                                nc.gpsimd.partition_broadcast(bcS[:], rtile[:])
                                ysl = yT_all[j][64 * e : 64 * e + 64, q0 : q0 + TS]
                                nc.vector.tensor_mul(
                                    ysl, ysl, bcS[64 * e : 64 * e + 64, :]
                                )

                # ---------------- out projection ----------------
                with (
                    tc.tile_pool(name="wopool", bufs=1) as wop,
                    tc.tile_pool(name="ostage", bufs=4) as osp,
                    tc.tile_pool(name="psW", bufs=4, space="PSUM") as psW,
                ):
                    wot = []
                    for cc in range(4):
                        w = wop.tile([128, C], f32r, tag=f"wo{cc}", name=f"wo{cc}")
                        nc.sync.dma_start(
                            w[:], woT[128 * cc : 128 * (cc + 1), :].bitcast(f32r)
                        )
                        wot.append(w)
                    for ts in range(NTS):
                        for co in range(8):
                            p = psW.tile([128, TS], f32, tag="op", name=f"o{ts}{co}")
                            for cc in range(4):
                                nc.tensor.matmul(
                                    p[:],
                                    wot[cc][:, 128 * co : 128 * (co + 1)],
                                    yT_all[cc][:, TS * ts : TS * (ts + 1)],
                                    start=(cc == 0),
                                    stop=(cc == 3),
                                )
                            o = osp.tile([128, TS], f32, tag="os", name=f"os{ts}{co}")
                            nc.scalar.copy(o[:], p[:])
                            nc.sync.dma_start(
                                outT[
                                    128 * co : 128 * (co + 1), TS * ts : TS * (ts + 1)
                                ],
                                o[:],
                            )

    nc.compile()
    return nc


def _get_program():
    if "nc" not in _CACHE:
        _CACHE["nc"] = _build_program()
    return _CACHE["nc"]


def _host_inputs(x, cos, sin, Wqkv, Wo):
    """Build the 8 per-core input maps."""
    # permutation of one head-section's 512 rows (head-relative):
    # row-tile layout [x1 h0-3 | x2 h0-3 | x1 h4-7 | x2 h4-7], 32 rows/block
    perm = []
    for grp in range(2):
        for half in range(2):
            for lh in range(4 * grp, 4 * grp + 4):
                for jj in range(32):
                    perm.append(64 * lh + 2 * jj + half)
    perm = np.asarray(perm)

    cosT4 = np.ascontiguousarray(np.tile(cos.T, (4, 1)).astype(np.float32))
    sinT4 = np.ascontiguousarray(np.tile(sin.T, (4, 1)).astype(np.float32))

    # mask [128, 512+128]: 512 zero cols then a lower-triangular 128 block
    tri = (np.arange(128)[:, None] <= np.arange(128)[None, :]).astype(np.float32)
    maskd = np.ascontiguousarray(
        np.concatenate([np.zeros((128, TS), np.float32), tri], axis=1)
    )

    in_maps = []
    for c in range(NCORES):
        b, g = c // 2, c % 2
        hs0 = HPC * g
        sec = np.arange(QR) + DH * hs0  # this core's rows within a section
        Wq = Wqkv[sec[perm], :]
        Wk = Wqkv[C + sec[perm], :]
        Wv = Wqkv[2 * C + sec, :]
        wqkvT = np.ascontiguousarray(np.concatenate([Wq, Wk, Wv], 0).T)
        woTc = np.ascontiguousarray(Wo[:, sec].T)
        xTb = np.ascontiguousarray(x[b].T)
        in_maps.append(
            {
                "xT": xTb,
                "wqkvT": wqkvT,
                "woT": woTc,
                "cosT": cosT4,
                "sinT": sinT4,
                "maskd": maskd,
            }
        )
    return in_maps


def kernel(x, cos, sin, Wqkv, Wo, _want_profile=False):
    from concourse.bass_utils import run_bass_kernel_spmd

    x = np.asarray(x, dtype=np.float32)
    cos = np.asarray(cos, dtype=np.float32)
    sin = np.asarray(sin, dtype=np.float32)
    Wqkv = np.asarray(Wqkv, dtype=np.float32)
    Wo = np.asarray(Wo, dtype=np.float32)

    nc = _get_program()
    in_maps = _host_inputs(x, cos, sin, Wqkv, Wo)
    kw = {}
    if _want_profile:
        import os, shutil

        tmpdir = "/tmp/bass_trace"
        shutil.rmtree(tmpdir, ignore_errors=True)
        os.makedirs(tmpdir, exist_ok=True)
        kw["tmpdir"] = tmpdir
    res = run_bass_kernel_spmd(
        nc, in_maps, list(range(NCORES)), trace=_want_profile, **kw
    )
    out = np.empty((B, T, C), dtype=np.float32)
    for b in range(B):
        acc = (
            res.results[2 * b]["outT"].astype(np.float32)
            + res.results[2 * b + 1]["outT"].astype(np.float32)
        )
        out[b] = acc.T
    if _want_profile:
        return out, res
    return out



# revision 36
# speedup vs baseline: 1.0980x; 1.0980x over previous
"""Multi-head attention (RoPE + causal softmax + out-proj) on 8 TRN2 NeuronCores.

Sharding: core c handles batch b = c // 2 and head-half g = c % 2 (8 of 16
heads). Each core computes q/k/v projections for its heads, RoPE, causal
attention, and a partial transposed output projection
outT = (y_heads @ Wo_part.T).T; the host sums the two partials per batch.

v2 design notes (driven by NTFF trace of the fp32 baseline):
 - All matmul operands are bf16 (PE rate equals fp32r, but LDWEIGHTS gets
   fast-weight-load, DMA bytes halve, DVE gets 2x modes). PSUM stays fp32.
 - Attention q-chunks are 256 wide: causal tile waste drops to 59.4% and
   PSUM needs only 6 banks (sT double-buffered 2x2 + 2 y banks), leaving 2
   banks to overlap the first half of the output projection into attention.
 - Scores run as K=32 halves on 4 concurrent 32-row PE tiles (proven
   concurrent in the baseline trace).
 - Causal masking via gpsimd.affine_select on the 128-wide diagonal strip
   only; fully-masked halves of diagonal tiles are never exp'd (dedicated
   eT buffers keep those columns zero).
 - exp on ScalarE is the attention bottleneck (~126us); everything else is
   arranged to keep the PE >85% busy so the HAM clock gate stays at 8/8.
 - Output projection: pass 1 (heads 0-3) interleaved into the second head
   group's attention; pass 2 (heads 4-7) + add at the end.
"""

import numpy as np

B, T, C, H = 4, 2048, 1024, 16
DH = C // H  # 64
NCORES = 8
HPC = H // 2  # 8 heads per core
QR = HPC * DH  # 512 rows per q/k/v section
TS = 512  # projection t-chunk width
NTS = T // TS  # 4
CC = C // 128  # 8 contraction chunks
NKT = T // 128  # 16 k-tiles
QC = 256  # attention q-chunk width
NQC = T // QC  # 8

_CACHE = {}


def _build_program():
    import concourse.mybir as mybir
    import concourse.tile as tile
    from concourse import bacc

    f32 = mybir.dt.float32
    bf16 = mybir.dt.bfloat16
    EXP = mybir.ActivationFunctionType.Exp
    GE = mybir.AluOpType.is_ge

    nc = bacc.Bacc(trn_type="TRN2")

    xT = nc.dram_tensor("xT", [C, T], bf16, kind="ExternalInput").ap()
    wqkvT = nc.dram_tensor("wqkvT", [C, 3 * QR], bf16, kind="ExternalInput").ap()
    woT = nc.dram_tensor("woT", [QR, C], bf16, kind="ExternalInput").ap()
    cosT = nc.dram_tensor("cosT", [128, T], f32, kind="ExternalInput").ap()
    sinT = nc.dram_tensor("sinT", [128, T], f32, kind="ExternalInput").ap()
    outT = nc.dram_tensor("outT", [C, T], f32, kind="ExternalOutput").ap()
    o1d = nc.dram_tensor("o1d", [C, T], bf16, kind="Internal").ap()

    import os

    _dbg = bool(os.environ.get("KDBG"))
    if _dbg:
        dbg_qk = [
            nc.dram_tensor(f"dbg_qk{p}{g}{h}", [128, T], bf16, kind="ExternalOutput").ap()
            for p in range(2) for g in range(2) for h in range(2)
        ]
        dbg_va = nc.dram_tensor("dbg_va", [128, HPC * 65], bf16, kind="ExternalOutput").ap()
        dbg_dn = nc.dram_tensor("dbg_dn", [2, 2 * NQC * 512], f32, kind="ExternalOutput").ap()
        dbg_yT = [
            nc.dram_tensor(f"dbg_yT{j}", [128, T], bf16, kind="ExternalOutput").ap()
            for j in range(4)
        ]
        dbg_e = [
            nc.dram_tensor(f"dbg_e{i}", [128, 512], bf16, kind="ExternalOutput").ap()
            for i in range(6)
        ]
        dbg_yu = nc.dram_tensor("dbg_yu", [128, 1024], bf16, kind="ExternalOutput").ap()

    with tile.TileContext(nc) as tc:
        with tc.tile_pool(name="persist", bufs=1) as pp:
            # rope'd q/k in projection layout: tile (part, grp, half):
            # rows 32*i = x-half of local head 4*grp+i
            qk = {}
            for part in range(2):
                for grp in range(2):
                    for half in range(2):
                        nm = f"qk{part}{grp}{half}"
                        qk[(part, grp, half)] = pp.tile(
                            [128, T], bf16, tag=nm, name=nm
                        )
            # v with a ones column per head: [t-chunk 128, 8 * 65]
            v_aug = [
                pp.tile([128, HPC * 65], bf16, tag=f"va{t}", name=f"va{t}")
                for t in range(NKT)
            ]
            yT_all = [
                pp.tile([128, T], bf16, tag=f"ya{j}", name=f"ya{j}")
                for j in range(4)
            ]
            # denominators per head group: row = head pair (2 used),
            # cols [qi(8), pair-half(2), 256]
            dng = [
                pp.tile([2, NQC * 512], f32, tag=f"dn{g}", name=f"dn{g}")
                for g in range(2)
            ]

            # ---------------- phase A: qkv projection + rope ----------------
            with (
                tc.tile_pool(name="wpool", bufs=1) as wp,
                tc.tile_pool(name="xpool", bufs=9) as xp,
                tc.tile_pool(name="trig", bufs=1) as tp,
                tc.tile_pool(name="ropetmp", bufs=6) as rt,
                tc.tile_pool(name="psA", bufs=4, space="PSUM") as psA,
                tc.tile_pool(name="psWarm", bufs=1, space="PSUM") as psWm,
            ):
                # PE warmup: keep the HAM clock-gate busy while first DMAs land
                warm = tp.tile([128, TS], bf16, tag="warm")
                nc.gpsimd.memset(warm[:], 0.0)
                wps = psWm.tile([128, TS], f32, tag="warmp")
                for i in range(20):
                    nc.tensor.matmul(wps[:], warm[:, 0:128], warm[:], start=True, stop=True)

                # first weight chunk, then first x chunk set, then the rest
                wtiles = [None] * CC
                w = wp.tile([128, 3 * QR], bf16, tag="w0", name="w0")
                nc.sync.dma_start(w[:], wqkvT[0:128, :])
                wtiles[0] = w
                xts0 = []
                for cc in range(CC):
                    xt = xp.tile([128, TS], bf16, tag="xts", name=f"x0{cc}")
                    nc.sync.dma_start(xt[:], xT[128 * cc : 128 * (cc + 1), 0:TS])
                    xts0.append(xt)
                for cc in range(1, CC):
                    w = wp.tile([128, 3 * QR], bf16, tag=f"w{cc}", name=f"w{cc}")
                    nc.sync.dma_start(w[:], wqkvT[128 * cc : 128 * (cc + 1), :])
                    wtiles[cc] = w
                ct = tp.tile([128, T], f32, tag="cos")
                st_ = tp.tile([128, T], f32, tag="sin")
                nc.sync.dma_start(ct[:], cosT[:])
                nc.sync.dma_start(st_[:], sinT[:])

                ones8 = tp.tile([128, HPC], bf16, tag="ones8")
                nc.gpsimd.memset(ones8[:], 1.0)
                for t in range(NKT):
                    var = v_aug[t].rearrange("p (h d) -> p h d", h=HPC)
                    nc.vector.tensor_copy(var[:, :, 64:65], ones8[:].unsqueeze(2))

                for ts in range(NTS):
                    if ts == 0:
                        xts = xts0
                    else:
                        xts = []
                        for cc in range(CC):
                            xt = xp.tile([128, TS], bf16, tag="xts", name=f"x{ts}{cc}")
                            nc.sync.dma_start(
                                xt[:],
                                xT[128 * cc : 128 * (cc + 1), TS * ts : TS * (ts + 1)],
                            )
                            xts.append(xt)

                    # q/k rows -> rope (written straight into persistent tiles)
                    for part in range(2):  # 0=q, 1=k
                        for grp in range(2):  # local heads 4*grp .. 4*grp+3
                            ptiles = []
                            for half in range(2):  # x1, x2
                                p = psA.tile(
                                    [128, TS], f32, tag="proj", name=f"p{ts}{part}{grp}{half}"
                                )
                                col0 = QR * part + 256 * grp + 128 * half
                                for cc in range(CC):
                                    nc.tensor.matmul(
                                        p[:],
                                        wtiles[cc][:, col0 : col0 + 128],
                                        xts[cc][:],
                                        start=(cc == 0),
                                        stop=(cc == CC - 1),
                                    )
                                ptiles.append(p)
                            x1p, x2p = ptiles
                            csl = ct[:, TS * ts : TS * (ts + 1)]
                            ssl = st_[:, TS * ts : TS * (ts + 1)]
                            o1 = qk[(part, grp, 0)][:, TS * ts : TS * (ts + 1)]
                            o2 = qk[(part, grp, 1)][:, TS * ts : TS * (ts + 1)]
                            t1 = rt.tile([128, TS], f32, tag="rt", name=f"t1{ts}{part}{grp}")
                            t2 = rt.tile([128, TS], f32, tag="rt", name=f"t2{ts}{part}{grp}")
                            nc.vector.tensor_mul(t1[:], x1p[:], csl)
                            nc.vector.tensor_mul(t2[:], x2p[:], ssl)
                            nc.vector.tensor_sub(o1, t1[:], t2[:])
                            t3 = rt.tile([128, TS], f32, tag="rt", name=f"t3{ts}{part}{grp}")
                            t4 = rt.tile([128, TS], f32, tag="rt", name=f"t4{ts}{part}{grp}")
                            nc.vector.tensor_mul(t3[:], x1p[:], ssl)
                            nc.vector.tensor_mul(t4[:], x2p[:], csl)
                            nc.vector.tensor_add(o2, t3[:], t4[:])

                    # v projection straight into v_aug
                    for tr4 in range(4):
                        t = 4 * ts + tr4
                        p = psA.tile([128, QR], f32, tag="proj", name=f"pv{ts}{tr4}")
                        for cc in range(CC):
                            nc.tensor.matmul(
                                p[:],
                                xts[cc][:, 128 * tr4 : 128 * (tr4 + 1)],
                                wtiles[cc][:, 2 * QR : 3 * QR],
                                start=(cc == 0),
                                stop=(cc == CC - 1),
                            )
                        var = v_aug[t].rearrange("p (h d) -> p h d", h=HPC)
                        nc.vector.tensor_copy(
                            var[:, :, 0:64],
                            p[:].rearrange("p (h d) -> p h d", h=HPC),
                        )

            # ---------------- phase B: attention (+ out-proj pass 1) --------
            with (
                tc.tile_pool(name="wopool", bufs=1) as wop,
                tc.tile_pool(name="o1pool", bufs=2) as o1p,
                tc.tile_pool(name="rcpool", bufs=1) as rcpool,
                tc.tile_pool(name="epool", bufs=4) as ep,
                tc.tile_pool(name="edpool", bufs=4) as edp,
                tc.tile_pool(name="bcpool", bufs=1) as bcp,
                tc.tile_pool(name="dstage", bufs=2) as dsp,
                tc.tile_pool(name="rstage", bufs=2) as rsp,
                tc.tile_pool(name="ostage", bufs=4) as osp,
                tc.tile_pool(name="psS", bufs=2, space="PSUM") as psS,
                tc.tile_pool(name="psY", bufs=1, space="PSUM") as psY,
                tc.tile_pool(name="psW", bufs=2, space="PSUM") as psW,
            ):
                wot = []
                for cc in range(4):
                    w = wop.tile([128, C], bf16, tag=f"wo{cc}", name=f"wo{cc}")
                    nc.sync.dma_start(w[:], woT[128 * cc : 128 * (cc + 1), :])
                    wot.append(w)

                # r=1 diagonal eT buffers: columns [0:128] per head stay zero
                ed1 = []
                for i in range(4):
                    e = edp.tile([128, 512], bf16, tag="ed1", name=f"ed1_{i}")
                    nc.gpsimd.memset(e[:], 0.0)
                    ed1.append(e)
                ed1_i = 0

                # out-proj pass-1 work items, interleaved into g4=1 attention;
                # partials bounce through DRAM (SBUF is tight)
                pass1_items = [(ts, co) for ts in range(NTS) for co in range(8)]
                pass1_pos = 0
                o1_dmas = {}

                def emit_pass1():
                    nonlocal pass1_pos
                    if pass1_pos >= len(pass1_items):
                        return
                    ts, co = pass1_items[pass1_pos]
                    pass1_pos += 1
                    p = psW.tile([128, TS], f32, tag="op", name=f"o1p{ts}{co}")
                    for cc in range(2):
                        nc.tensor.matmul(
                            p[:],
                            wot[cc][:, 128 * co : 128 * (co + 1)],
                            yT_all[cc][:, TS * ts : TS * (ts + 1)],
                            start=(cc == 0),
                            stop=(cc == 1),
                        )
                    o = o1p.tile([128, TS], bf16, tag="o1", name=f"o1_{ts}{co}")
                    nc.vector.tensor_copy(o[:], p[:])
                    o1_dmas[(ts, co)] = nc.sync.dma_start(
                        o1d[128 * co : 128 * (co + 1), TS * ts : TS * (ts + 1)], o[:]
                    )

                for g4 in range(2):  # head groups of 4: heads 4*g4..4*g4+3
                    for qi in range(NQC):
                        q0 = QC * qi
                        nkt = 2 * (qi + 1)
                        # two y banks: bank b = head pair, [headA | headB] cols
                        yb = [
                            psY.tile([65, 512], f32, tag=f"yb{b}", name=f"yb{g4}_{qi}_{b}")
                            for b in range(2)
                        ]
                        def emit_pv(kt, evs):
                            for sub in range(2):
                                for hh in range(2):
                                    h = 4 * g4 + 2 * sub + hh
                                    # start=True clears has_written for the
                                    # WHOLE bank: only the first matmul into
                                    # the bank may set it. hh=1's kt=0 write
                                    # lands fresh because the bank was just
                                    # cleared by hh=0's start.
                                    nc.tensor.matmul(
                                        yb[sub][:, 256 * hh : 256 * (hh + 1)],
                                        v_aug[kt][:, 65 * h : 65 * h + 65],
                                        evs[sub][:, hh, :],
                                        start=(kt == 0 and hh == 0),
                                        stop=(kt == nkt - 1),
                                        skip_group_check=True,
                                    )

                        prev_pv = None  # (kt, [ev_sub0, ev_sub1])
                        for kt in range(nkt):
                            k0 = 128 * kt
                            r = kt - 2 * qi  # 0 or 1 => diagonal tiles
                            lo = 128 * max(r, 0)
                            evs = []
                            for sub in range(2):  # head pair {2sub, 2sub+1}
                                # sT: even head cols 0:256 (bank a), odd head
                                # cols 512:768 (bank b) — concurrent row-tiles
                                # never share a PSUM bank
                                sT = psS.tile(
                                    [128, 1024], f32, tag="sT",
                                    name=f"sT{g4}_{qi}_{kt}_{sub}",
                                )
                                for hh in range(2):
                                    lh4 = 2 * sub + hh
                                    rb = 32 * lh4
                                    for half in range(2):
                                        nc.tensor.matmul(
                                            sT[:, 512 * hh : 512 * hh + QC],
                                            qk[(1, g4, half)][rb : rb + 32, k0 : k0 + 128],
                                            qk[(0, g4, half)][rb : rb + 32, q0 : q0 + QC],
                                            start=(half == 0),
                                            stop=(half == 1),
                                            tile_position=(rb, 0),
                                        )
                                if r == 1:
                                    eT = ed1[ed1_i]
                                    ed1_i = (ed1_i + 1) % 4
                                else:
                                    eT = ep.tile(
                                        [128, 512], bf16, tag="eT",
                                        name=f"eT{g4}_{qi}_{kt}_{sub}",
                                    )
                                ev = eT.rearrange("p (s q) -> p s q", s=2)
                                sv = sT.rearrange("p (s q) -> p s q", s=2)
                                nc.scalar.activation(
                                    ev[:, :, lo:QC], sv[:, :, lo:QC], EXP, scale=0.125
                                )
                                if r >= 0:
                                    # triangular strip: keep where qcol >= kpart
                                    nc.gpsimd.affine_select(
                                        ev[:, :, lo : lo + 128],
                                        ev[:, :, lo : lo + 128],
                                        pattern=[[0, 2], [1, 128]],
                                        compare_op=GE,
                                        fill=0.0,
                                        base=0,
                                        channel_multiplier=-1,
                                    )
                                evs.append(ev)
                                if _dbg and g4 == 0 and qi <= 1 and sub == 0:
                                    di = kt if qi == 0 else 2 + kt
                                    nc.sync.dma_start(dbg_e[di][:], eT[:])
                            # PV delayed one kt: scores(kt) issue ahead of
                            # PV(kt-1) so the PE never waits on exp(kt)
                            if prev_pv is not None:
                                emit_pv(*prev_pv)
                            prev_pv = (kt, evs)
                            if g4 == 1 and kt % 2 == 0:
                                emit_pass1()
                        emit_pv(*prev_pv)

                        # row done: stage denominators + unnormalized copy out
                        for b in range(2):
                            dtmp = dsp.tile(
                                [65, 512], f32, tag="dt", name=f"dt{g4}_{qi}_{b}"
                            )
                            nc.vector.tensor_copy(dtmp[64:65, :], yb[b][64:65, :])
                            nc.sync.dma_start(
                                dng[g4][b : b + 1, 512 * qi : 512 * (qi + 1)],
                                dtmp[64:65, :],
                            )
                        for lh4 in range(4):
                            h = 4 * g4 + lh4
                            j, e_ = h // 2, h % 2
                            b = lh4 // 2
                            nc.vector.tensor_copy(
                                yT_all[j][64 * e_ : 64 * e_ + 64, q0 : q0 + QC],
                                yb[b][0:64, 256 * (lh4 % 2) : 256 * (lh4 % 2) + 256],
                            )
                        if _dbg and g4 == 0 and qi == 1:
                            nc.sync.dma_start(dbg_yu[:], yT_all[0][:, 0:1024])

                    # per-group normalization (overlaps next group's attention)
                    rcps = rcpool.tile(
                        [2, NQC * 512], f32, tag="rcps", name=f"rcps{g4}"
                    )
                    nc.vector.reciprocal(rcps[:], dng[g4][:])
                    dnv = rcps.rearrange("p (qi b c) -> p qi b c", qi=NQC, b=2)
                    for lh4 in range(4):
                        h = 4 * g4 + lh4
                        j, e_ = h // 2, h % 2
                        pr = lh4 // 2
                        hb = lh4 % 2
                        # broadcast input must start at partition 0: DMA-stage
                        rh = rsp.tile([1, T], f32, tag="rh", name=f"rh{h}")
                        rv = rh.rearrange("p (qi one c) -> p qi one c", qi=NQC, one=1)
                        nc.sync.dma_start(
                            rv[:, :, :, :], dnv[pr : pr + 1, :, hb : hb + 1, :]
                        )
                        bcS = bcp.tile([128, T], f32, tag="bb", name=f"bb{h}")
                        nc.gpsimd.partition_broadcast(bcS[:], rh[:])
                        ysl = yT_all[j][64 * e_ : 64 * e_ + 64, :]
                        nc.vector.tensor_mul(ysl, ysl, bcS[64 * e_ : 64 * e_ + 64, :])

                # drain any remaining pass-1 items
                while pass1_pos < len(pass1_items):
                    emit_pass1()

                # ---------------- out projection pass 2 ----------------
                from concourse.tile_rust import add_dep_helper

                for ts in range(NTS):
                    for co in range(8):
                        oin = o1p.tile(
                            [128, TS], bf16, tag="o1in", name=f"o1in{ts}{co}"
                        )
                        din = nc.sync.dma_start(
                            oin[:],
                            o1d[128 * co : 128 * (co + 1), TS * ts : TS * (ts + 1)],
                        )
                        # DRAM RAW: the framework doesn't track deps through
                        # dram tensors — order the read-back after the write
                        add_dep_helper(
                            din.ins, o1_dmas[(ts, co)].ins, reason="o1d RAW"
                        )
                        p = psW.tile([128, TS], f32, tag="op", name=f"o2p{ts}{co}")
                        for cc in range(2, 4):
                            nc.tensor.matmul(
                                p[:],
                                wot[cc][:, 128 * co : 128 * (co + 1)],
                                yT_all[cc][:, TS * ts : TS * (ts + 1)],
                                start=(cc == 2),
                                stop=(cc == 3),
                            )
                        o = osp.tile([128, TS], f32, tag="os", name=f"os{ts}{co}")
                        nc.vector.tensor_add(o[:], p[:], oin[:])
                        nc.sync.dma_start(
                            outT[
                                128 * co : 128 * (co + 1), TS * ts : TS * (ts + 1)
                            ],
                            o[:],
                        )

            if _dbg:
                i = 0
                for p_ in range(2):
                    for g_ in range(2):
                        for h_ in range(2):
                            nc.sync.dma_start(dbg_qk[i][:], qk[(p_, g_, h_)][:])
                            i += 1
                nc.sync.dma_start(dbg_va[:], v_aug[0][:])
                for g_ in range(2):
                    nc.sync.dma_start(
                        dbg_dn[:, g_ * NQC * 512 : (g_ + 1) * NQC * 512],
                        dng[g_][:],
                    )
                for j in range(4):
                    nc.sync.dma_start(dbg_yT[j][:], yT_all[j][:])

    nc.compile()
    return nc


def _get_program():
    if "nc" not in _CACHE:
        _CACHE["nc"] = _build_program()
    return _CACHE["nc"]


def _host_inputs(x, cos, sin, Wqkv, Wo):
    """Build the 8 per-core input maps."""
    import ml_dtypes

    bf16 = ml_dtypes.bfloat16
    # permutation of one head-section's 512 rows (head-relative):
    # row-tile layout [x1 h0-3 | x2 h0-3 | x1 h4-7 | x2 h4-7], 32 rows/block
    perm = []
    for grp in range(2):
        for half in range(2):
            for lh in range(4 * grp, 4 * grp + 4):
                for jj in range(32):
                    perm.append(64 * lh + 2 * jj + half)
    perm = np.asarray(perm)

    cosT4 = np.ascontiguousarray(np.tile(cos.T, (4, 1)).astype(np.float32))
    sinT4 = np.ascontiguousarray(np.tile(sin.T, (4, 1)).astype(np.float32))

    in_maps = []
    for c in range(NCORES):
        b, g = c // 2, c % 2
        hs0 = HPC * g
        sec = np.arange(QR) + DH * hs0  # this core's rows within a section
        Wq = Wqkv[sec[perm], :]
        Wk = Wqkv[C + sec[perm], :]
        Wv = Wqkv[2 * C + sec, :]
        wqkvT = np.ascontiguousarray(np.concatenate([Wq, Wk, Wv], 0).T).astype(bf16)
        woTc = np.ascontiguousarray(Wo[:, sec].T).astype(bf16)
        xTb = np.ascontiguousarray(x[b].T).astype(bf16)
        in_maps.append(
            {
                "xT": xTb,
                "wqkvT": wqkvT,
                "woT": woTc,
                "cosT": cosT4,
                "sinT": sinT4,
            }
        )
    return in_maps


def kernel(x, cos, sin, Wqkv, Wo, _want_profile=False):
    from concourse.bass_utils import run_bass_kernel_spmd

    x = np.asarray(x, dtype=np.float32)
    cos = np.asarray(cos, dtype=np.float32)
    sin = np.asarray(sin, dtype=np.float32)
    Wqkv = np.asarray(Wqkv, dtype=np.float32)
    Wo = np.asarray(Wo, dtype=np.float32)

    nc = _get_program()
    in_maps = _host_inputs(x, cos, sin, Wqkv, Wo)
    kw = {}
    if _want_profile:
        import os, shutil

        tmpdir = "/tmp/bass_trace"
        shutil.rmtree(tmpdir, ignore_errors=True)
        os.makedirs(tmpdir, exist_ok=True)
        kw["tmpdir"] = tmpdir
    res = run_bass_kernel_spmd(
        nc, in_maps, list(range(NCORES)), trace=_want_profile, **kw
    )
    out = np.empty((B, T, C), dtype=np.float32)
    for b in range(B):
        acc = (
            res.results[2 * b]["outT"].astype(np.float32)
            + res.results[2 * b + 1]["outT"].astype(np.float32)
        )
        out[b] = acc.T
    if _want_profile:
        return out, res
    return out


# revision 39
# speedup vs baseline: 1.4837x; 1.3512x over previous
"""Multi-head attention (RoPE + causal softmax + out-proj) on 8 TRN2 NeuronCores.

Sharding: core c handles batch b = c // 2 and head-half g = c % 2 (8 of 16
heads). Each core computes q/k/v projections for its heads, RoPE, causal
attention, and a partial transposed output projection
outT = (y_heads @ Wo_part.T).T; the host sums the two partials per batch.

v2 design notes (driven by NTFF trace of the fp32 baseline):
 - All matmul operands are bf16 (PE rate equals fp32r, but LDWEIGHTS gets
   fast-weight-load, DMA bytes halve, DVE gets 2x modes). PSUM stays fp32.
 - Attention q-chunks are 256 wide: causal tile waste drops to 59.4% and
   PSUM needs only 6 banks (sT double-buffered 2x2 + 2 y banks), leaving 2
   banks to overlap the first half of the output projection into attention.
 - Scores run as K=32 halves on 4 concurrent 32-row PE tiles (proven
   concurrent in the baseline trace).
 - Causal masking via gpsimd.affine_select on the 128-wide diagonal strip
   only; fully-masked halves of diagonal tiles are never exp'd (dedicated
   eT buffers keep those columns zero).
 - exp on ScalarE is the attention bottleneck (~126us); everything else is
   arranged to keep the PE >85% busy so the HAM clock gate stays at 8/8.
 - Output projection: pass 1 (heads 0-3) interleaved into the second head
   group's attention; pass 2 (heads 4-7) + add at the end.
"""

import numpy as np

B, T, C, H = 4, 2048, 1024, 16
DH = C // H  # 64
NCORES = 8
HPC = H // 2  # 8 heads per core
QR = HPC * DH  # 512 rows per q/k/v section
TS = 512  # projection t-chunk width
NTS = T // TS  # 4
CC = C // 128  # 8 contraction chunks
NKT = T // 128  # 16 k-tiles
QC = 256  # attention q-chunk width
NQC = T // QC  # 8

_CACHE = {}


def _build_program():
    import concourse.mybir as mybir
    import concourse.tile as tile
    from concourse import bacc

    f32 = mybir.dt.float32
    bf16 = mybir.dt.bfloat16
    EXP = mybir.ActivationFunctionType.Exp
    GE = mybir.AluOpType.is_ge

    nc = bacc.Bacc(trn_type="TRN2")

    xT = nc.dram_tensor("xT", [C, T], bf16, kind="ExternalInput").ap()
    wqkvT = nc.dram_tensor("wqkvT", [C, 3 * QR], bf16, kind="ExternalInput").ap()
    woT = nc.dram_tensor("woT", [QR, C], bf16, kind="ExternalInput").ap()
    cosT = nc.dram_tensor("cosT", [128, T], f32, kind="ExternalInput").ap()
    sinT = nc.dram_tensor("sinT", [128, T], f32, kind="ExternalInput").ap()
    outT = nc.dram_tensor("outT", [C, T], f32, kind="ExternalOutput").ap()
    o1d = nc.dram_tensor("o1d", [C, T], bf16, kind="Internal").ap()

    import os

    _dbg = bool(os.environ.get("KDBG"))
    if _dbg:
        dbg_qk = [
            nc.dram_tensor(f"dbg_qk{p}{g}{h}", [128, T], bf16, kind="ExternalOutput").ap()
            for p in range(2) for g in range(2) for h in range(2)
        ]
        dbg_va = nc.dram_tensor("dbg_va", [128, HPC * 65], bf16, kind="ExternalOutput").ap()
        dbg_dn = nc.dram_tensor("dbg_dn", [2, 2 * NQC * 512], f32, kind="ExternalOutput").ap()
        dbg_yT = [
            nc.dram_tensor(f"dbg_yT{j}", [128, T], bf16, kind="ExternalOutput").ap()
            for j in range(4)
        ]
        dbg_e = [
            nc.dram_tensor(f"dbg_e{i}", [128, 512], bf16, kind="ExternalOutput").ap()
            for i in range(6)
        ]
        dbg_yu = nc.dram_tensor("dbg_yu", [128, 1024], bf16, kind="ExternalOutput").ap()

    with tile.TileContext(nc) as tc:
        with tc.tile_pool(name="persist", bufs=1) as pp:
            # rope'd q/k in projection layout: tile (part, grp, half):
            # rows 32*i = x-half of local head 4*grp+i
            qk = {}
            for part in range(2):
                for grp in range(2):
                    for half in range(2):
                        nm = f"qk{part}{grp}{half}"
                        qk[(part, grp, half)] = pp.tile(
                            [128, T], bf16, tag=nm, name=nm
                        )
            # v with a ones column per head: [t-chunk 128, 8 * 65]
            v_aug = [
                pp.tile([128, HPC * 65], bf16, tag=f"va{t}", name=f"va{t}")
                for t in range(NKT)
            ]
            yT_all = [
                pp.tile([128, T], bf16, tag=f"ya{j}", name=f"ya{j}")
                for j in range(4)
            ]
            # denominators per head group: row = head pair (2 used),
            # cols [qi(8), pair-half(2), 256]
            dng = [
                pp.tile([2, NQC * 512], f32, tag=f"dn{g}", name=f"dn{g}")
                for g in range(2)
            ]

            # ---------------- phase A: qkv projection + rope ----------------
            with (
                tc.tile_pool(name="wpool", bufs=1) as wp,
                tc.tile_pool(name="xpool", bufs=9) as xp,
                tc.tile_pool(name="trig", bufs=1) as tp,
                tc.tile_pool(name="ropetmp", bufs=6) as rt,
                tc.tile_pool(name="psA", bufs=4, space="PSUM") as psA,
                tc.tile_pool(name="psWarm", bufs=1, space="PSUM") as psWm,
            ):
                # PE warmup: keep the HAM clock-gate busy while first DMAs land
                warm = tp.tile([128, TS], bf16, tag="warm")
                nc.gpsimd.memset(warm[:], 0.0)
                wps = psWm.tile([128, TS], f32, tag="warmp")
                for i in range(20):
                    nc.tensor.matmul(wps[:], warm[:, 0:128], warm[:], start=True, stop=True)

                # first weight chunk, then first x chunk set, then the rest
                wtiles = [None] * CC
                w = wp.tile([128, 3 * QR], bf16, tag="w0", name="w0")
                nc.sync.dma_start(w[:], wqkvT[0:128, :])
                wtiles[0] = w
                xts0 = []
                for cc in range(CC):
                    xt = xp.tile([128, TS], bf16, tag="xts", name=f"x0{cc}")
                    nc.sync.dma_start(xt[:], xT[128 * cc : 128 * (cc + 1), 0:TS])
                    xts0.append(xt)
                for cc in range(1, CC):
                    w = wp.tile([128, 3 * QR], bf16, tag=f"w{cc}", name=f"w{cc}")
                    nc.sync.dma_start(w[:], wqkvT[128 * cc : 128 * (cc + 1), :])
                    wtiles[cc] = w
                ct = tp.tile([128, T], f32, tag="cos")
                st_ = tp.tile([128, T], f32, tag="sin")
                nc.sync.dma_start(ct[:], cosT[:])
                nc.sync.dma_start(st_[:], sinT[:])

                ones8 = tp.tile([128, HPC], bf16, tag="ones8")
                nc.gpsimd.memset(ones8[:], 1.0)
                for t in range(NKT):
                    var = v_aug[t].rearrange("p (h d) -> p h d", h=HPC)
                    nc.vector.tensor_copy(var[:, :, 64:65], ones8[:].unsqueeze(2))

                for ts in range(NTS):
                    if ts == 0:
                        xts = xts0
                    else:
                        xts = []
                        for cc in range(CC):
                            xt = xp.tile([128, TS], bf16, tag="xts", name=f"x{ts}{cc}")
                            nc.sync.dma_start(
                                xt[:],
                                xT[128 * cc : 128 * (cc + 1), TS * ts : TS * (ts + 1)],
                            )
                            xts.append(xt)

                    # q/k rows -> rope (written straight into persistent tiles)
                    for part in range(2):  # 0=q, 1=k
                        for grp in range(2):  # local heads 4*grp .. 4*grp+3
                            ptiles = []
                            for half in range(2):  # x1, x2
                                p = psA.tile(
                                    [128, TS], f32, tag="proj", name=f"p{ts}{part}{grp}{half}"
                                )
                                col0 = QR * part + 256 * grp + 128 * half
                                for cc in range(CC):
                                    nc.tensor.matmul(
                                        p[:],
                                        wtiles[cc][:, col0 : col0 + 128],
                                        xts[cc][:],
                                        start=(cc == 0),
                                        stop=(cc == CC - 1),
                                    )
                                ptiles.append(p)
                            x1p, x2p = ptiles
                            csl = ct[:, TS * ts : TS * (ts + 1)]
                            ssl = st_[:, TS * ts : TS * (ts + 1)]
                            o1 = qk[(part, grp, 0)][:, TS * ts : TS * (ts + 1)]
                            o2 = qk[(part, grp, 1)][:, TS * ts : TS * (ts + 1)]
                            t1 = rt.tile([128, TS], f32, tag="rt", name=f"t1{ts}{part}{grp}")
                            t2 = rt.tile([128, TS], f32, tag="rt", name=f"t2{ts}{part}{grp}")
                            nc.vector.tensor_mul(t1[:], x1p[:], csl)
                            nc.vector.tensor_mul(t2[:], x2p[:], ssl)
                            nc.vector.tensor_sub(o1, t1[:], t2[:])
                            t3 = rt.tile([128, TS], f32, tag="rt", name=f"t3{ts}{part}{grp}")
                            t4 = rt.tile([128, TS], f32, tag="rt", name=f"t4{ts}{part}{grp}")
                            nc.vector.tensor_mul(t3[:], x1p[:], ssl)
                            nc.vector.tensor_mul(t4[:], x2p[:], csl)
                            nc.vector.tensor_add(o2, t3[:], t4[:])

                    # v projection straight into v_aug
                    for tr4 in range(4):
                        t = 4 * ts + tr4
                        p = psA.tile([128, QR], f32, tag="proj", name=f"pv{ts}{tr4}")
                        for cc in range(CC):
                            nc.tensor.matmul(
                                p[:],
                                xts[cc][:, 128 * tr4 : 128 * (tr4 + 1)],
                                wtiles[cc][:, 2 * QR : 3 * QR],
                                start=(cc == 0),
                                stop=(cc == CC - 1),
                            )
                        var = v_aug[t].rearrange("p (h d) -> p h d", h=HPC)
                        nc.vector.tensor_copy(
                            var[:, :, 0:64],
                            p[:].rearrange("p (h d) -> p h d", h=HPC),
                        )

            # ---------------- phase B: attention (+ out-proj pass 1) --------
            with (
                tc.tile_pool(name="wopool", bufs=1) as wop,
                tc.tile_pool(name="o1pool", bufs=2) as o1p,
                tc.tile_pool(name="rcpool", bufs=1) as rcpool,
                tc.tile_pool(name="epool", bufs=4) as ep,
                tc.tile_pool(name="edpool", bufs=4) as edp,
                tc.tile_pool(name="bcpool", bufs=1) as bcp,
                tc.tile_pool(name="dstage", bufs=2) as dsp,
                tc.tile_pool(name="rstage", bufs=2) as rsp,
                tc.tile_pool(name="ostage", bufs=4) as osp,
                tc.tile_pool(name="psS", bufs=2, space="PSUM") as psS,
                tc.tile_pool(name="psY", bufs=1, space="PSUM") as psY,
                tc.tile_pool(name="psW", bufs=2, space="PSUM") as psW,
            ):
                wot = []
                for cc in range(4):
                    w = wop.tile([128, C], bf16, tag=f"wo{cc}", name=f"wo{cc}")
                    nc.sync.dma_start(w[:], woT[128 * cc : 128 * (cc + 1), :])
                    wot.append(w)

                # r=1 diagonal eT buffers: columns [0:128] per head stay zero
                ed1 = []
                for i in range(4):
                    e = edp.tile([128, 512], bf16, tag="ed1", name=f"ed1_{i}")
                    nc.gpsimd.memset(e[:], 0.0)
                    ed1.append(e)
                ed1_i = 0

                # out-proj pass-1 work items, interleaved into g4=1 attention;
                # partials bounce through DRAM (SBUF is tight)
                pass1_items = [(ts, co) for ts in range(NTS) for co in range(8)]
                pass1_pos = 0
                o1_dmas = {}

                def emit_pass1():
                    nonlocal pass1_pos
                    if pass1_pos >= len(pass1_items):
                        return
                    ts, co = pass1_items[pass1_pos]
                    pass1_pos += 1
                    p = psW.tile([128, TS], f32, tag="op", name=f"o1p{ts}{co}")
                    for cc in range(2):
                        nc.tensor.matmul(
                            p[:],
                            wot[cc][:, 128 * co : 128 * (co + 1)],
                            yT_all[cc][:, TS * ts : TS * (ts + 1)],
                            start=(cc == 0),
                            stop=(cc == 1),
                        )
                    o = o1p.tile([128, TS], bf16, tag="o1", name=f"o1_{ts}{co}")
                    nc.vector.tensor_copy(o[:], p[:])
                    o1_dmas[(ts, co)] = nc.sync.dma_start(
                        o1d[128 * co : 128 * (co + 1), TS * ts : TS * (ts + 1)], o[:]
                    )

                for g4 in range(2):  # head groups of 4: heads 4*g4..4*g4+3
                    for qi in range(NQC):
                        q0 = QC * qi
                        nkt = 2 * (qi + 1)
                        # two y banks: bank b = head pair, [headA | headB] cols
                        yb = [
                            psY.tile([65, 512], f32, tag=f"yb{b}", name=f"yb{g4}_{qi}_{b}")
                            for b in range(2)
                        ]
                        def emit_pv(kt, evs):
                            for sub in range(2):
                                for hh in range(2):
                                    h = 4 * g4 + 2 * sub + hh
                                    # start=True clears has_written for the
                                    # WHOLE bank: only the first matmul into
                                    # the bank may set it. hh=1's kt=0 write
                                    # lands fresh because the bank was just
                                    # cleared by hh=0's start.
                                    nc.tensor.matmul(
                                        yb[sub][:, 256 * hh : 256 * (hh + 1)],
                                        v_aug[kt][:, 65 * h : 65 * h + 65],
                                        evs[sub][:, hh, :],
                                        start=(kt == 0 and hh == 0),
                                        stop=(kt == nkt - 1),
                                        skip_group_check=True,
                                    )

                        prev_pv = None  # (kt, [ev_sub0, ev_sub1])
                        for kt in range(nkt):
                            k0 = 128 * kt
                            r = kt - 2 * qi  # 0 or 1 => diagonal tiles
                            lo = 128 * max(r, 0)
                            evs = []
                            for sub in range(2):  # head pair {2sub, 2sub+1}
                                # sT: even head cols 0:256 (bank a), odd head
                                # cols 512:768 (bank b) — concurrent row-tiles
                                # never share a PSUM bank
                                sT = psS.tile(
                                    [128, 1024], f32, tag="sT",
                                    name=f"sT{g4}_{qi}_{kt}_{sub}",
                                )
                                for hh in range(2):
                                    lh4 = 2 * sub + hh
                                    rb = 32 * lh4
                                    for half in range(2):
                                        nc.tensor.matmul(
                                            sT[:, 512 * hh : 512 * hh + QC],
                                            qk[(1, g4, half)][rb : rb + 32, k0 : k0 + 128],
                                            qk[(0, g4, half)][rb : rb + 32, q0 : q0 + QC],
                                            start=(half == 0),
                                            stop=(half == 1),
                                            tile_position=(rb, 0),
                                        )
                                if r == 1:
                                    eT = ed1[ed1_i]
                                    ed1_i = (ed1_i + 1) % 4
                                else:
                                    eT = ep.tile(
                                        [128, 512], bf16, tag="eT",
                                        name=f"eT{g4}_{qi}_{kt}_{sub}",
                                    )
                                ev = eT.rearrange("p (s q) -> p s q", s=2)
                                sv = sT.rearrange("p (s q) -> p s q", s=2)
                                nc.scalar.activation(
                                    ev[:, :, lo:QC], sv[:, :, lo:QC], EXP, scale=0.125
                                )
                                if r >= 0:
                                    # triangular strip: keep where qcol >= kpart
                                    nc.gpsimd.affine_select(
                                        ev[:, :, lo : lo + 128],
                                        ev[:, :, lo : lo + 128],
                                        pattern=[[0, 2], [1, 128]],
                                        compare_op=GE,
                                        fill=0.0,
                                        base=0,
                                        channel_multiplier=-1,
                                    )
                                evs.append(ev)
                                if _dbg and g4 == 0 and qi <= 1 and sub == 0:
                                    di = kt if qi == 0 else 2 + kt
                                    nc.sync.dma_start(dbg_e[di][:], eT[:])
                            # PV delayed one kt: scores(kt) issue ahead of
                            # PV(kt-1) so the PE never waits on exp(kt)
                            if prev_pv is not None:
                                emit_pv(*prev_pv)
                            prev_pv = (kt, evs)
                            # only from qi>=2: pass1 depends on g4=0's
                            # normalization — emitting it earlier blocks the
                            # in-order PE stream on the norm chain
                            if g4 == 1 and qi >= 2 and kt % 2 == 0:
                                emit_pass1()
                        emit_pv(*prev_pv)

                        # row done: stage denominators + unnormalized copy out
                        for b in range(2):
                            dtmp = dsp.tile(
                                [65, 512], f32, tag="dt", name=f"dt{g4}_{qi}_{b}"
                            )
                            nc.vector.tensor_copy(dtmp[64:65, :], yb[b][64:65, :])
                            nc.sync.dma_start(
                                dng[g4][b : b + 1, 512 * qi : 512 * (qi + 1)],
                                dtmp[64:65, :],
                            )
                        for lh4 in range(4):
                            h = 4 * g4 + lh4
                            j, e_ = h // 2, h % 2
                            b = lh4 // 2
                            nc.vector.tensor_copy(
                                yT_all[j][64 * e_ : 64 * e_ + 64, q0 : q0 + QC],
                                yb[b][0:64, 256 * (lh4 % 2) : 256 * (lh4 % 2) + 256],
                            )
                        if _dbg and g4 == 0 and qi == 1:
                            nc.sync.dma_start(dbg_yu[:], yT_all[0][:, 0:1024])

                    # per-group normalization (overlaps next group's attention)
                    rcps = rcpool.tile(
                        [2, NQC * 512], f32, tag="rcps", name=f"rcps{g4}"
                    )
                    nc.vector.reciprocal_approx_fast(rcps[:], dng[g4][:])
                    dnv = rcps.rearrange("p (qi b c) -> p qi b c", qi=NQC, b=2)
                    for lh4 in range(4):
                        h = 4 * g4 + lh4
                        j, e_ = h // 2, h % 2
                        pr = lh4 // 2
                        hb = lh4 % 2
                        # broadcast input must start at partition 0: DMA-stage
                        rh = rsp.tile([1, T], f32, tag="rh", name=f"rh{h}")
                        rv = rh.rearrange("p (qi one c) -> p qi one c", qi=NQC, one=1)
                        nc.sync.dma_start(
                            rv[:, :, :, :], dnv[pr : pr + 1, :, hb : hb + 1, :]
                        )
                        bcS = bcp.tile([128, T], f32, tag="bb", name=f"bb{h}")
                        nc.gpsimd.partition_broadcast(bcS[:], rh[:])
                        ysl = yT_all[j][64 * e_ : 64 * e_ + 64, :]
                        nc.vector.tensor_mul(ysl, ysl, bcS[64 * e_ : 64 * e_ + 64, :])

                # drain any remaining pass-1 items
                while pass1_pos < len(pass1_items):
                    emit_pass1()

                # ---------------- out projection pass 2 ----------------
                # software-pipelined: o1 read-backs prefetched 6 deep so the
                # DMA → matmul → add → DMA chain doesn't serialize per item
                from concourse.tile_rust import add_dep_helper

                p2items = [(ts, co) for ts in range(NTS) for co in range(8)]
                PF = 6
                oins = {}
                for i in range(len(p2items) + PF):
                    if i < len(p2items):
                        ts, co = p2items[i]
                        oin = o1p.tile(
                            [128, TS], bf16, tag="o1in", name=f"o1in{ts}{co}",
                            bufs=PF + 2,
                        )
                        din = nc.sync.dma_start(
                            oin[:],
                            o1d[128 * co : 128 * (co + 1), TS * ts : TS * (ts + 1)],
                        )
                        # DRAM RAW: deps aren't tracked through dram tensors
                        add_dep_helper(
                            din.ins, o1_dmas[(ts, co)].ins, reason="o1d RAW"
                        )
                        oins[(ts, co)] = oin
                    j = i - PF
                    if j >= 0:
                        ts, co = p2items[j]
                        p = psW.tile([128, TS], f32, tag="op", name=f"o2p{ts}{co}")
                        for cc in range(2, 4):
                            nc.tensor.matmul(
                                p[:],
                                wot[cc][:, 128 * co : 128 * (co + 1)],
                                yT_all[cc][:, TS * ts : TS * (ts + 1)],
                                start=(cc == 2),
                                stop=(cc == 3),
                            )
                        o = osp.tile([128, TS], f32, tag="os", name=f"os{ts}{co}")
                        nc.vector.tensor_add(o[:], p[:], oins[(ts, co)][:])
                        nc.sync.dma_start(
                            outT[
                                128 * co : 128 * (co + 1), TS * ts : TS * (ts + 1)
                            ],
                            o[:],
                        )

            if _dbg:
                i = 0
                for p_ in range(2):
                    for g_ in range(2):
                        for h_ in range(2):
                            nc.sync.dma_start(dbg_qk[i][:], qk[(p_, g_, h_)][:])
                            i += 1
                nc.sync.dma_start(dbg_va[:], v_aug[0][:])
                for g_ in range(2):
                    nc.sync.dma_start(
                        dbg_dn[:, g_ * NQC * 512 : (g_ + 1) * NQC * 512],
                        dng[g_][:],
                    )
                for j in range(4):
                    nc.sync.dma_start(dbg_yT[j][:], yT_all[j][:])

    nc.compile()
    return nc


def _get_program():
    if "nc" not in _CACHE:
        _CACHE["nc"] = _build_program()
    return _CACHE["nc"]


def _host_inputs(x, cos, sin, Wqkv, Wo):
    """Build the 8 per-core input maps."""
    import ml_dtypes

    bf16 = ml_dtypes.bfloat16
    # permutation of one head-section's 512 rows (head-relative):
    # row-tile layout [x1 h0-3 | x2 h0-3 | x1 h4-7 | x2 h4-7], 32 rows/block
    perm = []
    for grp in range(2):
        for half in range(2):
            for lh in range(4 * grp, 4 * grp + 4):
                for jj in range(32):
                    perm.append(64 * lh + 2 * jj + half)
    perm = np.asarray(perm)

    cosT4 = np.ascontiguousarray(np.tile(cos.T, (4, 1)).astype(np.float32))
    sinT4 = np.ascontiguousarray(np.tile(sin.T, (4, 1)).astype(np.float32))

    in_maps = []
    for c in range(NCORES):
        b, g = c // 2, c % 2
        hs0 = HPC * g
        sec = np.arange(QR) + DH * hs0  # this core's rows within a section
        Wq = Wqkv[sec[perm], :]
        Wk = Wqkv[C + sec[perm], :]
        Wv = Wqkv[2 * C + sec, :]
        wqkvT = np.ascontiguousarray(np.concatenate([Wq, Wk, Wv], 0).T).astype(bf16)
        woTc = np.ascontiguousarray(Wo[:, sec].T).astype(bf16)
        xTb = np.ascontiguousarray(x[b].T).astype(bf16)
        in_maps.append(
            {
                "xT": xTb,
                "wqkvT": wqkvT,
                "woT": woTc,
                "cosT": cosT4,
                "sinT": sinT4,
            }
        )
    return in_maps


def kernel(x, cos, sin, Wqkv, Wo, _want_profile=False):
    from concourse.bass_utils import run_bass_kernel_spmd

    x = np.asarray(x, dtype=np.float32)
    cos = np.asarray(cos, dtype=np.float32)
    sin = np.asarray(sin, dtype=np.float32)
    Wqkv = np.asarray(Wqkv, dtype=np.float32)
    Wo = np.asarray(Wo, dtype=np.float32)

    nc = _get_program()
    in_maps = _host_inputs(x, cos, sin, Wqkv, Wo)
    kw = {}
    if _want_profile:
        import os, shutil

        tmpdir = "/tmp/bass_trace"
        shutil.rmtree(tmpdir, ignore_errors=True)
        os.makedirs(tmpdir, exist_ok=True)
        kw["tmpdir"] = tmpdir
    res = run_bass_kernel_spmd(
        nc, in_maps, list(range(NCORES)), trace=_want_profile, **kw
    )
    out = np.empty((B, T, C), dtype=np.float32)
    for b in range(B):
        acc = (
            res.results[2 * b]["outT"].astype(np.float32)
            + res.results[2 * b + 1]["outT"].astype(np.float32)
        )
        out[b] = acc.T
    if _want_profile:
        return out, res
    return out


# revision 49
# speedup vs baseline: 1.6165x; 1.0895x over previous
"""Multi-head attention (RoPE + causal softmax + out-proj) on 8 TRN2 NeuronCores.

Sharding: core c handles batch b = c // 2 and head-half g = c % 2 (8 of 16
heads). Each core computes q/k/v projections for its heads, RoPE, causal
attention, and a partial transposed output projection
outT = (y_heads @ Wo_part.T).T; the host sums the two partials per batch.

v2 design notes (driven by NTFF trace of the fp32 baseline):
 - All matmul operands are bf16 (PE rate equals fp32r, but LDWEIGHTS gets
   fast-weight-load, DMA bytes halve, DVE gets 2x modes). PSUM stays fp32.
 - Attention q-chunks are 256 wide: causal tile waste drops to 59.4% and
   PSUM needs only 6 banks (sT double-buffered 2x2 + 2 y banks), leaving 2
   banks to overlap the first half of the output projection into attention.
 - Scores run as K=32 halves on 4 concurrent 32-row PE tiles (proven
   concurrent in the baseline trace).
 - Causal masking via gpsimd.affine_select on the 128-wide diagonal strip
   only; fully-masked halves of diagonal tiles are never exp'd (dedicated
   eT buffers keep those columns zero).
 - exp on ScalarE is the attention bottleneck (~126us); everything else is
   arranged to keep the PE >85% busy so the HAM clock gate stays at 8/8.
 - Output projection: pass 1 (heads 0-3) interleaved into the second head
   group's attention; pass 2 (heads 4-7) + add at the end.
"""

import numpy as np

B, T, C, H = 4, 2048, 1024, 16
DH = C // H  # 64
NCORES = 8
HPC = H // 2  # 8 heads per core
QR = HPC * DH  # 512 rows per q/k/v section
TS = 512  # projection t-chunk width
NTS = T // TS  # 4
CC = C // 128  # 8 contraction chunks
NKT = T // 128  # 16 k-tiles
QC = 256  # attention q-chunk width
NQC = T // QC  # 8

_CACHE = {}


def _build_program():
    import concourse.mybir as mybir
    import concourse.tile as tile
    from concourse import bacc

    f32 = mybir.dt.float32
    bf16 = mybir.dt.bfloat16
    EXP = mybir.ActivationFunctionType.Exp
    GE = mybir.AluOpType.is_ge

    nc = bacc.Bacc(trn_type="TRN2")

    xT = nc.dram_tensor("xT", [C, T], bf16, kind="ExternalInput").ap()
    wqkvT = nc.dram_tensor("wqkvT", [C, 3 * QR], bf16, kind="ExternalInput").ap()
    woT = nc.dram_tensor("woT", [QR, C], bf16, kind="ExternalInput").ap()
    cosT = nc.dram_tensor("cosT", [128, T], f32, kind="ExternalInput").ap()
    sinT = nc.dram_tensor("sinT", [128, T], f32, kind="ExternalInput").ap()
    outT = nc.dram_tensor("outT", [C, T], f32, kind="ExternalOutput").ap()

    import os

    _dbg = bool(os.environ.get("KDBG"))
    if _dbg:
        dbg_qk = [
            nc.dram_tensor(f"dbg_qk{p}{g}{h}", [128, T], bf16, kind="ExternalOutput").ap()
            for p in range(2) for g in range(2) for h in range(2)
        ]
        dbg_va = nc.dram_tensor("dbg_va", [128, HPC * 65], bf16, kind="ExternalOutput").ap()
        dbg_yT = [
            nc.dram_tensor(f"dbg_yT{j}", [128, T], bf16, kind="ExternalOutput").ap()
            for j in range(4)
        ]
        dbg_e = [
            nc.dram_tensor(f"dbg_e{i}", [128, 512], bf16, kind="ExternalOutput").ap()
            for i in range(6)
        ]
        dbg_yu = nc.dram_tensor("dbg_yu", [128, 1024], bf16, kind="ExternalOutput").ap()

    with tile.TileContext(nc) as tc:
        with tc.tile_pool(name="persist", bufs=1) as pp:
            # rope'd q/k in projection layout: tile (part, grp, half):
            # rows 32*i = x-half of local head 4*grp+i
            qk = {}
            for part in range(2):
                for grp in range(2):
                    for half in range(2):
                        nm = f"qk{part}{grp}{half}"
                        qk[(part, grp, half)] = pp.tile(
                            [128, T], bf16, tag=nm, name=nm
                        )
            # v with a ones column per head: [t-chunk 128, 8 * 65]
            v_aug = [
                pp.tile([128, HPC * 65], bf16, tag=f"va{t}", name=f"va{t}")
                for t in range(NKT)
            ]
            yT_all = [
                pp.tile([128, T], bf16, tag=f"ya{j}", name=f"ya{j}")
                for j in range(4)
            ]
            # denominators per head group: row = head pair (2 used),
            # cols [qi(8), pair-half(2), 256]


            # ---------------- phase A: qkv projection + rope ----------------
            with (
                tc.tile_pool(name="wpool", bufs=1) as wp,
                tc.tile_pool(name="xpool", bufs=9) as xp,
                tc.tile_pool(name="trig", bufs=1) as tp,
                tc.tile_pool(name="ropetmp", bufs=6) as rt,
                tc.tile_pool(name="psA", bufs=4, space="PSUM") as psA,
                tc.tile_pool(name="psWarm", bufs=1, space="PSUM") as psWm,
            ):
                # PE warmup: keep the HAM clock-gate busy while first DMAs land
                warm = tp.tile([128, TS], bf16, tag="warm")
                nc.gpsimd.memset(warm[:], 0.0)
                wps = psWm.tile([128, TS], f32, tag="warmp")
                for i in range(20):
                    nc.tensor.matmul(wps[:], warm[:, 0:128], warm[:], start=True, stop=True)

                # first weight chunk, then first x chunk set, then the rest
                wtiles = [None] * CC
                w = wp.tile([128, 3 * QR], bf16, tag="w0", name="w0")
                nc.sync.dma_start(w[:], wqkvT[0:128, :])
                wtiles[0] = w
                xts0 = []
                for cc in range(CC):
                    xt = xp.tile([128, TS], bf16, tag="xts", name=f"x0{cc}")
                    nc.sync.dma_start(xt[:], xT[128 * cc : 128 * (cc + 1), 0:TS])
                    xts0.append(xt)
                for cc in range(1, CC):
                    w = wp.tile([128, 3 * QR], bf16, tag=f"w{cc}", name=f"w{cc}")
                    nc.sync.dma_start(w[:], wqkvT[128 * cc : 128 * (cc + 1), :])
                    wtiles[cc] = w
                ct = tp.tile([128, T], f32, tag="cos")
                st_ = tp.tile([128, T], f32, tag="sin")
                nc.sync.dma_start(ct[:], cosT[:])
                nc.sync.dma_start(st_[:], sinT[:])

                ones8 = tp.tile([128, HPC], bf16, tag="ones8")
                nc.gpsimd.memset(ones8[:], 1.0)
                for t in range(NKT):
                    var = v_aug[t].rearrange("p (h d) -> p h d", h=HPC)
                    nc.vector.tensor_copy(var[:, :, 64:65], ones8[:].unsqueeze(2))

                for ts in range(NTS):
                    if ts == 0:
                        xts = xts0
                    else:
                        xts = []
                        for cc in range(CC):
                            xt = xp.tile([128, TS], bf16, tag="xts", name=f"x{ts}{cc}")
                            nc.sync.dma_start(
                                xt[:],
                                xT[128 * cc : 128 * (cc + 1), TS * ts : TS * (ts + 1)],
                            )
                            xts.append(xt)

                    # q/k rows -> rope (written straight into persistent tiles)
                    for part in range(2):  # 0=q, 1=k
                        for grp in range(2):  # local heads 4*grp .. 4*grp+3
                            ptiles = []
                            for half in range(2):  # x1, x2
                                p = psA.tile(
                                    [128, TS], f32, tag="proj", name=f"p{ts}{part}{grp}{half}"
                                )
                                col0 = QR * part + 256 * grp + 128 * half
                                for cc in range(CC):
                                    nc.tensor.matmul(
                                        p[:],
                                        wtiles[cc][:, col0 : col0 + 128],
                                        xts[cc][:],
                                        start=(cc == 0),
                                        stop=(cc == CC - 1),
                                    )
                                ptiles.append(p)
                            x1p, x2p = ptiles
                            csl = ct[:, TS * ts : TS * (ts + 1)]
                            ssl = st_[:, TS * ts : TS * (ts + 1)]
                            o1 = qk[(part, grp, 0)][:, TS * ts : TS * (ts + 1)]
                            o2 = qk[(part, grp, 1)][:, TS * ts : TS * (ts + 1)]
                            t1 = rt.tile([128, TS], f32, tag="rt", name=f"t1{ts}{part}{grp}")
                            t2 = rt.tile([128, TS], f32, tag="rt", name=f"t2{ts}{part}{grp}")
                            nc.vector.tensor_mul(t1[:], x1p[:], csl)
                            nc.vector.tensor_mul(t2[:], x2p[:], ssl)
                            nc.vector.tensor_sub(o1, t1[:], t2[:])
                            t3 = rt.tile([128, TS], f32, tag="rt", name=f"t3{ts}{part}{grp}")
                            t4 = rt.tile([128, TS], f32, tag="rt", name=f"t4{ts}{part}{grp}")
                            nc.vector.tensor_mul(t3[:], x1p[:], ssl)
                            nc.vector.tensor_mul(t4[:], x2p[:], csl)
                            nc.vector.tensor_add(o2, t3[:], t4[:])

                    # v projection straight into v_aug
                    for tr4 in range(4):
                        t = 4 * ts + tr4
                        p = psA.tile([128, QR], f32, tag="proj", name=f"pv{ts}{tr4}")
                        for cc in range(CC):
                            nc.tensor.matmul(
                                p[:],
                                xts[cc][:, 128 * tr4 : 128 * (tr4 + 1)],
                                wtiles[cc][:, 2 * QR : 3 * QR],
                                start=(cc == 0),
                                stop=(cc == CC - 1),
                            )
                        var = v_aug[t].rearrange("p (h d) -> p h d", h=HPC)
                        nc.vector.tensor_copy(
                            var[:, :, 0:64],
                            p[:].rearrange("p (h d) -> p h d", h=HPC),
                        )

            # ---------------- phase B: attention (+ out-proj pass 1) --------
            with (
                tc.tile_pool(name="wopool", bufs=1) as wop,
                tc.tile_pool(name="o1pool", bufs=1) as o1p,
                tc.tile_pool(name="epool", bufs=4) as ep,
                tc.tile_pool(name="edpool", bufs=4) as edp,
                tc.tile_pool(name="bcpool", bufs=2) as bcp,
                tc.tile_pool(name="dstage", bufs=2) as dsp,
                tc.tile_pool(name="rstage", bufs=2) as rsp,
                tc.tile_pool(name="ostage", bufs=4) as osp,
                tc.tile_pool(name="psS", bufs=2, space="PSUM") as psS,
                tc.tile_pool(name="psY", bufs=1, space="PSUM") as psY,
                tc.tile_pool(name="psW", bufs=2, space="PSUM") as psW,
            ):
                wot = []
                for cc in range(4):
                    w = wop.tile([128, C], bf16, tag=f"wo{cc}", name=f"wo{cc}")
                    nc.sync.dma_start(w[:], woT[128 * cc : 128 * (cc + 1), :])
                    wot.append(w)

                # r=1 diagonal eT buffers: columns [0:128] per head stay zero
                ed1 = []
                for i in range(4):
                    e = edp.tile([128, 512], bf16, tag="ed1", name=f"ed1_{i}")
                    nc.gpsimd.memset(e[:], 0.0)
                    ed1.append(e)
                ed1_i = 0

                # out-proj pass-1 work items, interleaved into g4=1 attention
                pass1_items = [(ts, co) for ts in range(NTS) for co in range(8)]
                pass1_pos = 0
                o1tiles = {}

                def emit_pass1():
                    nonlocal pass1_pos
                    if pass1_pos >= len(pass1_items):
                        return
                    ts, co = pass1_items[pass1_pos]
                    pass1_pos += 1
                    p = psW.tile([128, TS], f32, tag="op", name=f"o1p{ts}{co}")
                    for cc in range(2):
                        nc.tensor.matmul(
                            p[:],
                            wot[cc][:, 128 * co : 128 * (co + 1)],
                            yT_all[cc][:, TS * ts : TS * (ts + 1)],
                            start=(cc == 0),
                            stop=(cc == 1),
                        )
                    o = o1p.tile(
                        [128, TS], bf16, tag=f"o1_{ts}{co}", name=f"o1_{ts}{co}"
                    )
                    nc.vector.tensor_copy(o[:], p[:])
                    o1tiles[(ts, co)] = o

                for g4 in range(2):  # head groups of 4: heads 4*g4..4*g4+3
                    for qi in range(NQC):
                        q0 = QC * qi
                        nkt = 2 * (qi + 1)
                        # two y banks: bank b = head pair, [headA | headB] cols
                        yb = [
                            psY.tile([65, 512], f32, tag=f"yb{b}", name=f"yb{g4}_{qi}_{b}")
                            for b in range(2)
                        ]
                        def emit_pv(kt, evs):
                            for sub in range(2):
                                for hh in range(2):
                                    h = 4 * g4 + 2 * sub + hh
                                    # start=True clears has_written for the
                                    # WHOLE bank: only the first matmul into
                                    # the bank may set it. hh=1's kt=0 write
                                    # lands fresh because the bank was just
                                    # cleared by hh=0's start.
                                    nc.tensor.matmul(
                                        yb[sub][:, 256 * hh : 256 * (hh + 1)],
                                        v_aug[kt][:, 65 * h : 65 * h + 65],
                                        evs[sub][:, hh, :],
                                        start=(kt == 0 and hh == 0),
                                        stop=(kt == nkt - 1),
                                        skip_group_check=True,
                                    )

                        prev_pv = None  # (kt, [ev_sub0, ev_sub1])
                        for kt in range(nkt):
                            k0 = 128 * kt
                            r = kt - 2 * qi  # 0 or 1 => diagonal tiles
                            lo = 128 * max(r, 0)
                            evs = []
                            for sub in range(2):  # head pair {2sub, 2sub+1}
                                # sT: even head cols 0:256 (bank a), odd head
                                # cols 512:768 (bank b) — concurrent row-tiles
                                # never share a PSUM bank
                                sT = psS.tile(
                                    [128, 1024], f32, tag="sT",
                                    name=f"sT{g4}_{qi}_{kt}_{sub}",
                                )
                                for hh in range(2):
                                    lh4 = 2 * sub + hh
                                    rb = 32 * lh4
                                    for half in range(2):
                                        nc.tensor.matmul(
                                            sT[:, 512 * hh : 512 * hh + QC],
                                            qk[(1, g4, half)][rb : rb + 32, k0 : k0 + 128],
                                            qk[(0, g4, half)][rb : rb + 32, q0 : q0 + QC],
                                            start=(half == 0),
                                            stop=(half == 1),
                                            tile_position=(rb, 0),
                                        )
                                if r == 1:
                                    eT = ed1[ed1_i]
                                    ed1_i = (ed1_i + 1) % 4
                                else:
                                    eT = ep.tile(
                                        [128, 512], bf16, tag="eT",
                                        name=f"eT{g4}_{qi}_{kt}_{sub}",
                                    )
                                ev = eT.rearrange("p (s q) -> p s q", s=2)
                                sv = sT.rearrange("p (s q) -> p s q", s=2)
                                nc.scalar.activation(
                                    ev[:, :, lo:QC], sv[:, :, lo:QC], EXP, scale=0.125
                                )
                                if r >= 0:
                                    # triangular strip: keep where qcol >= kpart
                                    nc.gpsimd.affine_select(
                                        ev[:, :, lo : lo + 128],
                                        ev[:, :, lo : lo + 128],
                                        pattern=[[0, 2], [1, 128]],
                                        compare_op=GE,
                                        fill=0.0,
                                        base=0,
                                        channel_multiplier=-1,
                                    )
                                evs.append(ev)
                                if _dbg and g4 == 0 and qi <= 1 and sub == 0:
                                    di = kt if qi == 0 else 2 + kt
                                    nc.sync.dma_start(dbg_e[di][:], eT[:])
                            # PV delayed one kt: scores(kt) issue ahead of
                            # PV(kt-1) so the PE never waits on exp(kt)
                            if prev_pv is not None:
                                emit_pv(*prev_pv)
                            prev_pv = (kt, evs)
                            # g4=0 rows normalize per-row, so pass1 items are
                            # all unlocked once g4=1 begins
                            if g4 == 1 and kt % 2 == 0:
                                emit_pass1()
                        emit_pv(*prev_pv)

                        # row done: per-row normalization (overlaps later rows)
                        # den (yb row 64) -> partition 0 -> 1/den -> broadcast
                        for b in range(2):
                            dtmp = dsp.tile(
                                [65, 512], f32, tag="dt", name=f"dt{g4}_{qi}_{b}"
                            )
                            nc.vector.tensor_copy(dtmp[64:65, :], yb[b][64:65, :])
                            dr = rsp.tile([1, 512], f32, tag="dr", name=f"dr{g4}_{qi}_{b}")
                            nc.sync.dma_start(dr[:], dtmp[64:65, :])
                            rr = rsp.tile([1, 512], f32, tag="rr", name=f"rr{g4}_{qi}_{b}")
                            nc.vector.reciprocal_approx_fast(rr[:], dr[:])
                            bcS = bcp.tile(
                                [128, 512], f32, tag="bb", name=f"bb{g4}_{qi}_{b}"
                            )
                            nc.gpsimd.partition_broadcast(bcS[:], rr[:])
                            for hh in range(2):
                                lh4 = 2 * b + hh
                                h = 4 * g4 + lh4
                                j, e_ = h // 2, h % 2
                                # normalize during the copy-out: one DVE op
                                nc.vector.tensor_mul(
                                    yT_all[j][64 * e_ : 64 * e_ + 64, q0 : q0 + QC],
                                    yb[b][0:64, 256 * hh : 256 * hh + 256],
                                    bcS[0:64, 256 * hh : 256 * hh + 256],
                                )
                        if _dbg and g4 == 0 and qi == 1:
                            nc.sync.dma_start(dbg_yu[:], yT_all[0][:, 0:1024])

                # drain any remaining pass-1 items
                while pass1_pos < len(pass1_items):
                    emit_pass1()

                # ---------------- out projection pass 2 ----------------
                for ts in range(NTS):
                    for co in range(8):
                        p = psW.tile([128, TS], f32, tag="op", name=f"o2p{ts}{co}")
                        for cc in range(2, 4):
                            nc.tensor.matmul(
                                p[:],
                                wot[cc][:, 128 * co : 128 * (co + 1)],
                                yT_all[cc][:, TS * ts : TS * (ts + 1)],
                                start=(cc == 2),
                                stop=(cc == 3),
                            )
                        o = osp.tile([128, TS], f32, tag="os", name=f"os{ts}{co}")
                        nc.vector.tensor_add(o[:], p[:], o1tiles[(ts, co)][:])
                        nc.sync.dma_start(
                            outT[
                                128 * co : 128 * (co + 1), TS * ts : TS * (ts + 1)
                            ],
                            o[:],
                        )

            if _dbg:
                i = 0
                for p_ in range(2):
                    for g_ in range(2):
                        for h_ in range(2):
                            nc.sync.dma_start(dbg_qk[i][:], qk[(p_, g_, h_)][:])
                            i += 1
                nc.sync.dma_start(dbg_va[:], v_aug[0][:])
                for j in range(4):
                    nc.sync.dma_start(dbg_yT[j][:], yT_all[j][:])

    nc.compile()
    return nc


def _get_program():
    if "nc" not in _CACHE:
        _CACHE["nc"] = _build_program()
    return _CACHE["nc"]


def _host_inputs(x, cos, sin, Wqkv, Wo):
    """Build the 8 per-core input maps."""
    import ml_dtypes

    bf16 = ml_dtypes.bfloat16
    # permutation of one head-section's 512 rows (head-relative):
    # row-tile layout [x1 h0-3 | x2 h0-3 | x1 h4-7 | x2 h4-7], 32 rows/block
    perm = []
    for grp in range(2):
        for half in range(2):
            for lh in range(4 * grp, 4 * grp + 4):
                for jj in range(32):
                    perm.append(64 * lh + 2 * jj + half)
    perm = np.asarray(perm)

    cosT4 = np.ascontiguousarray(np.tile(cos.T, (4, 1)).astype(np.float32))
    sinT4 = np.ascontiguousarray(np.tile(sin.T, (4, 1)).astype(np.float32))

    in_maps = []
    for c in range(NCORES):
        b, g = c // 2, c % 2
        hs0 = HPC * g
        sec = np.arange(QR) + DH * hs0  # this core's rows within a section
        Wq = Wqkv[sec[perm], :]
        Wk = Wqkv[C + sec[perm], :]
        Wv = Wqkv[2 * C + sec, :]
        wqkvT = np.ascontiguousarray(np.concatenate([Wq, Wk, Wv], 0).T).astype(bf16)
        woTc = np.ascontiguousarray(Wo[:, sec].T).astype(bf16)
        xTb = np.ascontiguousarray(x[b].T).astype(bf16)
        in_maps.append(
            {
                "xT": xTb,
                "wqkvT": wqkvT,
                "woT": woTc,
                "cosT": cosT4,
                "sinT": sinT4,
            }
        )
    return in_maps


def kernel(x, cos, sin, Wqkv, Wo, _want_profile=False):
    from concourse.bass_utils import run_bass_kernel_spmd

    x = np.asarray(x, dtype=np.float32)
    cos = np.asarray(cos, dtype=np.float32)
    sin = np.asarray(sin, dtype=np.float32)
    Wqkv = np.asarray(Wqkv, dtype=np.float32)
    Wo = np.asarray(Wo, dtype=np.float32)

    nc = _get_program()
    in_maps = _host_inputs(x, cos, sin, Wqkv, Wo)
    kw = {}
    if _want_profile:
        import os, shutil

        tmpdir = "/tmp/bass_trace"
        shutil.rmtree(tmpdir, ignore_errors=True)
        os.makedirs(tmpdir, exist_ok=True)
        kw["tmpdir"] = tmpdir
    res = run_bass_kernel_spmd(
        nc, in_maps, list(range(NCORES)), trace=_want_profile, **kw
    )
    out = np.empty((B, T, C), dtype=np.float32)
    for b in range(B):
        acc = (
            res.results[2 * b]["outT"].astype(np.float32)
            + res.results[2 * b + 1]["outT"].astype(np.float32)
        )
        out[b] = acc.T
    if _want_profile:
        return out, res
    return out
